# revision 1
# baseline (speedup 1.0000x reference)
"""Cosine-similarity retrieval kernel for Trainium2 (Bass/Tile, 8 NeuronCores).

Computes sims[i] = dot(word_vectors[i], q) / ||word_vectors[i]|| with
q = inputs / ||inputs|| (query normalization folded in on the host).

Sharding: word_vectors row-sharded across 8 cores, query broadcast.
Each core processes R = 25088 rows (196 tiles of 128 rows); core 7's
slice overlaps core 6's by 704 rows so every core runs the identical
program (one NEFF), and the overlap rows compute bitwise-identical
values.

Per-core dataflow (memory-bound; HBM floor ~= 102.8 MB / ~360 GB/s):
  - rows are mapped to SBUF via the interleave  row = p*T + t
    (partition p in [0,128), tile t in [0,T)), so both the W loads and
    the final sims store are plain strided DMAs - no transpose anywhere.
  - per 128-row tile: one DVE tensor_tensor_reduce (elementwise mult
    with broadcast q + free-dim add-reduce, single 1x pass) for the dot,
    and one ACT activation(Square, accum_out) pass for the squared norm.
  - epilogue: norm = sqrt(norm2) (ACT), inv = 1/norm (DVE iterative
    divide), sims = dots * inv (DVE), one DMA out.
"""

import numpy as np

D = 1024          # embedding dim
N_FULL = 200000   # total rows
NCORES = 8
R = 25088         # rows per core = 128 * 196
T = R // 128      # 196 column-tiles per core
NT = 4            # tiles per DMA chunk (2 MiB per dma_start)
NCHUNK = T // NT  # 49

_NC_CACHE = {}


def _build_nc():
    if "nc" in _NC_CACHE:
        return _NC_CACHE["nc"]

    import concourse.tile as tile
    from concourse import bacc, mybir

    fp32 = mybir.dt.float32
    nc = bacc.Bacc(
        "TRN2",
        target_bir_lowering=False,
        debug=False,
        enable_asserts=False,
        num_devices=NCORES,
        enable_partition_id=False,
    )
    w = nc.dram_tensor("w", [R, D], fp32, kind="ExternalInput").ap()
    q = nc.dram_tensor("q", [D], fp32, kind="ExternalInput").ap()
    out = nc.dram_tensor("out", [R], fp32, kind="ExternalOutput").ap()

    # row p*T + t  <->  SBUF partition p, tile-column t
    w_v = w.rearrange("(p t) d -> p (t d)", p=128)  # [128, T*D], 4KB*T contig/part
    out_v = out.rearrange("(p t) -> p t", p=128)    # [128, T]

    with tile.TileContext(nc) as tc:
        with (
            tc.tile_pool(name="win", bufs=4) as win_pool,
            tc.tile_pool(name="aux", bufs=1) as aux_pool,
        ):
            qb = aux_pool.tile([128, D], fp32)
            nc.sync.dma_start(qb, q.partition_broadcast(128))

            dots = aux_pool.tile([128, T], fp32)
            norm2 = aux_pool.tile([128, T], fp32)
            scr_v = aux_pool.tile([128, D], fp32)
            scr_a = aux_pool.tile([128, D], fp32)

            for c in range(NCHUNK):
                wt = win_pool.tile([128, NT * D], fp32, name="wt")
                nc.sync.dma_start(wt, w_v[:, c * NT * D : (c + 1) * NT * D])
                for j in range(NT):
                    t = c * NT + j
                    sl = wt[:, j * D : (j + 1) * D]
                    # fused dot: scr = (sl*1+0)*qb, dots[:,t] = sum(scr).
                    # (TENSOR_TENSOR_REDUCE crashes this runtime; the
                    # custom-DVE affine_mul_reduce is the working fused
                    # multiply+reduce at the same 1x streaming rate.)
                    nc.vector.affine_mul_reduce(
                        out=scr_v,
                        accum_out=dots[:, t : t + 1],
                        in0=sl,
                        in1=qb,
                        scale=1.0,
                        bias=0.0,
                    )
                    nc.scalar.activation(
                        out=scr_a,
                        in_=sl,
                        func=mybir.ActivationFunctionType.Square,
                        accum_out=norm2[:, t : t + 1],
                    )

            norm = aux_pool.tile([128, T], fp32)
            nc.scalar.sqrt(norm, norm2)
            inv = aux_pool.tile([128, T], fp32)
            nc.vector.reciprocal(inv, norm)
            sims = aux_pool.tile([128, T], fp32)
            nc.vector.tensor_mul(sims, dots, inv)
            nc.sync.dma_start(out_v, sims)

    nc.compile()
    _NC_CACHE["nc"] = nc
    return nc


def _shard_starts():
    starts = [i * R for i in range(NCORES - 1)]
    starts.append(N_FULL - R)  # core 7 overlaps core 6 by 704 rows
    return starts


def make_in_maps(inputs: np.ndarray, word_vectors: np.ndarray):
    inputs = np.ascontiguousarray(inputs, dtype=np.float32)
    word_vectors = np.ascontiguousarray(word_vectors, dtype=np.float32)
    qn = inputs / np.maximum(np.linalg.norm(inputs), np.float32(1e-12))
    qn = qn.astype(np.float32)
    return [
        {"w": word_vectors[s : s + R], "q": qn} for s in _shard_starts()
    ]


def assemble(results) -> np.ndarray:
    full = np.empty(N_FULL, dtype=np.float32)
    for s, res in zip(_shard_starts(), results):
        full[s : s + R] = res["out"]
    return full


def kernel(inputs: np.ndarray, word_vectors: np.ndarray) -> np.ndarray:
    from concourse import bass_utils

    nc = _build_nc()
    in_maps = make_in_maps(inputs, word_vectors)
    res = bass_utils.run_bass_kernel_spmd(
        nc, in_maps, core_ids=list(range(NCORES))
    )
    return assemble(res.results)



# revision 2
# speedup vs baseline: 241.6278x; 241.6278x over previous
"""Cosine-similarity retrieval kernel for Trainium2 (Bass/Tile, 8 NeuronCores).

Computes sims[i] = dot(word_vectors[i], q) / ||word_vectors[i]|| with
q = inputs / ||inputs|| (query normalization folded in on the host).

Sharding: word_vectors row-sharded across 8 cores, query broadcast.
Each core processes R = 25088 rows (196 tiles of 128 rows); core 7's
slice overlaps core 6's by 704 rows so every core runs the identical
program (one NEFF), and the overlap rows compute bitwise-identical
values.

Per-core dataflow (memory-bound; HBM floor ~= 102.8 MB / ~360 GB/s
~= 286 us):
  - rows are mapped to SBUF via the interleave  row = p*T + t
    (partition p in [0,128), tile t in [0,T)), so both the W loads and
    the final sims store are plain strided DMAs - no transpose anywhere.
  - W streamed in 1 MiB chunks (NT=2 tiles, 8 KiB/partition); chunk
    DMAs are issued round-robin from the SP (sync) and Pool (gpsimd)
    queues.  A dma_start occupies its issuing engine's DGE queue for
    the whole transfer, so a single issuing engine serializes issue
    overhead (~625-900 ns/chunk) into the DMA stream; alternating two
    otherwise-idle engines lets one queue's transfer overlap the
    other's issue window (CoreSim: 320 us -> 252 us; HW is capped by
    HBM bandwidth ~286 us).
  - per 128-row tile: one DVE affine_mul_reduce (elementwise mult with
    broadcast q + free-dim add-reduce, single 1x pass) for the dot, and
    one ACT activation(Square, accum_out) pass for the squared norm.
    DVE busy ~221 us, ACT ~240 us: both hide under the DMA stream.
  - epilogue: norm = sqrt(norm2) (ACT), inv = 1/norm (DVE iterative
    divide), sims = dots * inv (DVE), one DMA out.
"""

import numpy as np

D = 1024          # embedding dim
N_FULL = 200000   # total rows
NCORES = 8
R = 25088         # rows per core = 128 * 196
T = R // 128      # 196 column-tiles per core
NT = 2            # tiles per DMA chunk (1 MiB per dma_start)
NCHUNK = T // NT  # 98

_NC_CACHE = {}


def _build_nc():
    if "nc" in _NC_CACHE:
        return _NC_CACHE["nc"]

    import concourse.tile as tile
    from concourse import bacc, mybir

    fp32 = mybir.dt.float32
    nc = bacc.Bacc(
        "TRN2",
        target_bir_lowering=False,
        debug=False,
        enable_asserts=False,
        num_devices=NCORES,
        enable_partition_id=False,
    )
    w = nc.dram_tensor("w", [R, D], fp32, kind="ExternalInput").ap()
    q = nc.dram_tensor("q", [D], fp32, kind="ExternalInput").ap()
    out = nc.dram_tensor("out", [R], fp32, kind="ExternalOutput").ap()

    # row p*T + t  <->  SBUF partition p, tile-column t
    w_v = w.rearrange("(p t) d -> p (t d)", p=128)  # [128, T*D], 4KB*T contig/part
    out_v = out.rearrange("(p t) -> p t", p=128)    # [128, T]

    with tile.TileContext(nc) as tc:
        with (
            tc.tile_pool(name="win", bufs=6) as win_pool,
            tc.tile_pool(name="aux", bufs=1) as aux_pool,
        ):
            qb = aux_pool.tile([128, D], fp32)
            nc.sync.dma_start(qb, q.partition_broadcast(128))

            dots = aux_pool.tile([128, T], fp32)
            norm2 = aux_pool.tile([128, T], fp32)
            scr_v = aux_pool.tile([128, D], fp32)
            scr_a = aux_pool.tile([128, D], fp32)

            dma_engines = (nc.gpsimd, nc.sync)
            for c in range(NCHUNK):
                wt = win_pool.tile([128, NT * D], fp32, name="wt")
                eng = dma_engines[c % len(dma_engines)]
                eng.dma_start(wt, w_v[:, c * NT * D : (c + 1) * NT * D])
                for j in range(NT):
                    t = c * NT + j
                    sl = wt[:, j * D : (j + 1) * D]
                    # fused dot: scr = (sl*1+0)*qb, dots[:,t] = sum(scr).
                    # (TENSOR_TENSOR_REDUCE crashes this runtime; the
                    # custom-DVE affine_mul_reduce is the working fused
                    # multiply+reduce at the same 1x streaming rate.)
                    nc.vector.affine_mul_reduce(
                        out=scr_v,
                        accum_out=dots[:, t : t + 1],
                        in0=sl,
                        in1=qb,
                        scale=1.0,
                        bias=0.0,
                    )
                    nc.scalar.activation(
                        out=scr_a,
                        in_=sl,
                        func=mybir.ActivationFunctionType.Square,
                        accum_out=norm2[:, t : t + 1],
                    )

            norm = aux_pool.tile([128, T], fp32)
            nc.scalar.sqrt(norm, norm2)
            inv = aux_pool.tile([128, T], fp32)
            nc.vector.reciprocal(inv, norm)
            sims = aux_pool.tile([128, T], fp32)
            nc.vector.tensor_mul(sims, dots, inv)
            nc.sync.dma_start(out_v, sims)

    nc.compile()
    _NC_CACHE["nc"] = nc
    return nc


def _shard_starts():
    starts = [i * R for i in range(NCORES - 1)]
    starts.append(N_FULL - R)  # core 7 overlaps core 6 by 704 rows
    return starts


def make_in_maps(inputs: np.ndarray, word_vectors: np.ndarray):
    inputs = np.ascontiguousarray(inputs, dtype=np.float32)
    word_vectors = np.ascontiguousarray(word_vectors, dtype=np.float32)
    qn = inputs / np.maximum(np.linalg.norm(inputs), np.float32(1e-12))
    qn = qn.astype(np.float32)
    return [
        {"w": word_vectors[s : s + R], "q": qn} for s in _shard_starts()
    ]


def assemble(results) -> np.ndarray:
    full = np.empty(N_FULL, dtype=np.float32)
    for s, res in zip(_shard_starts(), results):
        full[s : s + R] = res["out"]
    return full


def kernel(inputs: np.ndarray, word_vectors: np.ndarray) -> np.ndarray:
    from concourse import bass_utils

    nc = _build_nc()
    in_maps = make_in_maps(inputs, word_vectors)
    res = bass_utils.run_bass_kernel_spmd(
        nc, in_maps, core_ids=list(range(NCORES))
    )
    return assemble(res.results)
